# revision 14
# baseline (speedup 1.0000x reference)
"""Trainium2 Bass kernel for GAT(3 layers, 4 heads) + JK-LSTM + global pool + MLP.

Sharding: nodes (and their incoming edges) are partitioned across 8 NeuronCores.
Layer-0 node transform is computed replicated (input x is replicated); layers 1-2
exchange activations via AllGather. Segment softmax + scatter-add aggregation is
done with one-hot matmuls over destination-sorted edge chunks. The JK-LSTM,
attention, pooling and MLP are data-parallel over the node shard, with a final
AllReduce for the graph pooling.
"""
import os
import sys

for _p in ("/opt/trn_rl_repo", "/root/.axon_site/_ro/trn_rl_repo"):
    if os.path.isdir(_p) and _p not in sys.path:
        sys.path.append(_p)

import numpy as np

import concourse.bass as bass
import concourse.bacc as bacc
import concourse.mybir as mybir
import concourse.tile as tile

P = 128
N, E, G = 10000, 160000, 64
IN_C, HID, HEADS, OUT = 128, 128, 4, 8
C, L, HL = 512, 3, 768
NCORES = 8
NPC = N // NCORES          # 1250 nodes per core
NPCP = 1280                # padded (10 tiles of 128)
NTILES_OWN = NPCP // P     # 10
NT = NPCP * NCORES         # 10240 padded total
NTILES_ALL = NT // P       # 80
DH = 520                   # haug row: h(512) | a_s(4) | pad(4)
SENT = 255.0               # sentinel dest-local for padding edges

f32 = mybir.dt.float32
f32r = mybir.dt.float32r
i32 = mybir.dt.int32
AF = mybir.ActivationFunctionType
ALU = mybir.AluOpType

_CACHE = {}


def _gp(n):
    """node id -> padded global slot"""
    return (n // NPC) * NPCP + (n % NPC)


def build_tables(edge_index, batch):
    """Host-side preprocessing: destination-sorted, per-(core,tile) chunked edge
    tables, pooling matrix."""
    ei = np.concatenate(
        [np.asarray(edge_index), np.tile(np.arange(N, dtype=np.int32), (2, 1))], axis=1
    )
    s_arr, d_arr = ei[0].astype(np.int64), ei[1].astype(np.int64)
    own = d_arr // NPC
    per_ct = {}
    for k in range(NCORES):
        m = own == k
        sk, dk = s_arr[m], d_arr[m]
        dloc = dk - NPC * k
        t_all = dloc // P
        for t in range(NTILES_OWN):
            tm = t_all == t
            per_ct[(k, t)] = (sk[tm], dloc[tm] - t * P)
    nct = max((len(v[0]) + P - 1) // P for v in per_ct.values())
    srcidx = np.zeros((NCORES, P, NTILES_OWN * nct), np.int32)
    dlt = np.full((NCORES, P, NTILES_OWN * nct), SENT, np.float32)
    for k in range(NCORES):
        for t in range(NTILES_OWN):
            sk, dloc = per_ct[(k, t)]
            ne = len(sk)
            col0 = t * nct
            for c in range((ne + P - 1) // P):
                lo, hi = c * P, min((c + 1) * P, ne)
                srcidx[k, 0 : hi - lo, col0 + c] = _gp(sk[lo:hi])
                dlt[k, 0 : hi - lo, col0 + c] = dloc[lo:hi]
    batch = np.asarray(batch)
    cnt = np.maximum(np.bincount(batch, minlength=G), 1).astype(np.float32)
    poolmat = np.zeros((NCORES, P, NTILES_OWN, G), np.float32)
    inv = 1.0 / cnt
    for n in range(N):
        k, sl = n // NPC, n % NPC
        poolmat[k, sl % P, sl // P, batch[n]] = inv[batch[n]]
    return nct, srcidx, dlt, poolmat


def build_nc(nct):
    NCH = NTILES_OWN * nct
    nc = bacc.Bacc("TRN2", target_bir_lowering=False, debug=False, num_devices=NCORES)

    # ---------------- kernel I/O ----------------
    d_xT = nc.dram_tensor("xT", [P, NT], f32r, kind="ExternalInput")
    d_xTown = nc.dram_tensor("xTown", [P, NTILES_OWN, P], f32r, kind="ExternalInput")
    d_W = [
        nc.dram_tensor("W0d", [P, C], f32r, kind="ExternalInput"),
        nc.dram_tensor("W1d", [C, C], f32r, kind="ExternalInput"),
        nc.dram_tensor("W2d", [C, C], f32r, kind="ExternalInput"),
    ]
    d_asrc = [nc.dram_tensor(f"asrc{l}", [P, C], f32, kind="ExternalInput") for l in range(L)]
    d_adst = [nc.dram_tensor(f"adst{l}", [P, C], f32, kind="ExternalInput") for l in range(L)]
    d_brep = [nc.dram_tensor(f"brep{l}", [P, C], f32, kind="ExternalInput") for l in range(L)]
    d_srcidx = nc.dram_tensor("srcidx", [P, NCH], i32, kind="ExternalInput")
    d_dlt = nc.dram_tensor("dlt", [P, NCH], f32, kind="ExternalInput")
    d_wih = [nc.dram_tensor(f"WihT_{d}", [C, 4 * HL], f32r, kind="ExternalInput") for d in "fr"]
    d_whh = [nc.dram_tensor(f"WhhT_{d}", [HL, 4 * HL], f32r, kind="ExternalInput") for d in "fr"]
    d_bsum = nc.dram_tensor("bsum", [P, 48], f32, kind="ExternalInput")
    d_attw = nc.dram_tensor("attw", [P, 12], f32r, kind="ExternalInput")
    d_poolmat = nc.dram_tensor("poolmat", [P, NTILES_OWN, G], f32r, kind="ExternalInput")
    d_fc1 = nc.dram_tensor("fc1W", [C, C], f32r, kind="ExternalInput")
    d_fc2 = nc.dram_tensor("fc2W", [C, C], f32r, kind="ExternalInput")
    d_fc3 = nc.dram_tensor("fc3W", [C, OUT], f32r, kind="ExternalInput")
    d_fcb = nc.dram_tensor("fcb", [P, 8], f32, kind="ExternalInput")  # fc1_b | fc2_b
    d_fc3b = nc.dram_tensor("fc3b", [OUT, 1], f32, kind="ExternalInput")
    d_out = nc.dram_tensor("out_T", [OUT, G], f32, kind="ExternalOutput")

    # ---------------- internal DRAM ----------------
    d_haug = [
        nc.dram_tensor("haug0", [NT, DH], f32r),
        nc.dram_tensor("haug1", [NT, DH], f32r, addr_space="Shared"),
        nc.dram_tensor("haug2", [NT, DH], f32r, addr_space="Shared"),
    ]
    d_hsh = [None, nc.dram_tensor("hsh1", [NPCP, DH], f32r), nc.dram_tensor("hsh2", [NPCP, DH], f32r)]
    d_x = [nc.dram_tensor(f"x_l{l}", [NPCP, C], f32) for l in range(L)]
    d_xt = [nc.dram_tensor(f"xt_l{l}", [C, NPCP], f32r) for l in range(L)]
    d_scores = nc.dram_tensor("scoresd", [L, NPCP], f32)
    d_poolin = nc.dram_tensor("poolin", [G, C], f32)
    d_pooled = nc.dram_tensor("pooled", [G, C], f32, addr_space="Shared")

    RG = [list(range(NCORES))]
    BLKS = [(0, 512), (512, 512), (1024, 256)]  # node blocks of NPCP

    with tile.TileContext(nc) as tc, \
         nc.allow_low_precision(reason="float32r is 4-byte fp32 storage"):
        with tc.tile_pool(name="const", bufs=1) as const:
            ident_f = const.tile([P, P], f32)
            from concourse.masks import make_identity

            make_identity(nc, ident_f[:])
            ident = const.tile([P, P], f32r)
            nc.vector.tensor_copy(out=ident[:], in_=ident_f[:])
            iota_f = const.tile([P, P], f32)
            nc.gpsimd.iota(iota_f[:], pattern=[[1, P]], base=0, channel_multiplier=0,
                           allow_small_or_imprecise_dtypes=True)
            srcidx = const.tile([P, NCH], i32)
            nc.sync.dma_start(out=srcidx[:], in_=d_srcidx[:, :])
            dlt = const.tile([P, NCH], f32)
            nc.sync.dma_start(out=dlt[:], in_=d_dlt[:, :])
            adtab = [const.tile([P, 4 * NTILES_OWN], f32r, tag=f"adtab{_l}", name=f"adtab{_l}") for _l in range(L)]
            scores = const.tile([1, L * NPCP], f32)
            nc.vector.memset(scores[:], 0.0)

            # ================= stage A0: full h0 -> haug0 (replicated) ========
            with tc.tile_pool(name="a0", bufs=2) as a0p, \
                 tc.tile_pool(name="a0ps", bufs=4, space="PSUM") as a0ps:
                W0t = a0p.tile([P, C], f32r, tag="w0", bufs=1)
                nc.sync.dma_start(out=W0t[:], in_=d_W[0][:, :])
                asr0 = a0p.tile([P, C], f32, tag="asr", bufs=1)
                nc.sync.dma_start(out=asr0[:], in_=d_asrc[0][:, :])
                ads0 = a0p.tile([P, C], f32, tag="ads", bufs=1)
                nc.sync.dma_start(out=ads0[:], in_=d_adst[0][:, :])
                for nt in range(NTILES_ALL):
                    xt_t = a0p.tile([P, P], f32r, tag="xt")
                    nc.sync.dma_start(out=xt_t[:], in_=d_xT[:, nt * P : (nt + 1) * P])
                    ps = a0ps.tile([P, C], f32, tag="ps")
                    nc.tensor.matmul(out=ps[:], lhsT=xt_t[:], rhs=W0t[:], start=True, stop=True)
                    ht = a0p.tile([P, DH], f32r, tag="ht")
                    nc.scalar.copy(out=ht[:, 0:C], in_=ps[:])
                    tmp = a0p.tile([P, C], f32, tag="tmp")
                    nc.vector.tensor_tensor(out=tmp[:], in0=ps[:], in1=asr0[:], op=ALU.mult)
                    nc.vector.tensor_reduce(
                        out=ht[:, C : C + 4],
                        in_=tmp[:].rearrange("p (h c) -> p h c", h=HEADS),
                        axis=mybir.AxisListType.X, op=ALU.add)
                    nc.sync.dma_start(out=d_haug[0][nt * P : (nt + 1) * P, :], in_=ht[:])
                # own-tile a_d for layer 0
                xTown = a0p.tile([P, NTILES_OWN, P], f32r, tag="xtown", bufs=1)
                nc.sync.dma_start(out=xTown[:], in_=d_xTown[:, :, :])
                for j in range(NTILES_OWN):
                    ps = a0ps.tile([P, C], f32, tag="ps")
                    nc.tensor.matmul(out=ps[:], lhsT=xTown[:, j, :], rhs=W0t[:], start=True, stop=True)
                    tmp = a0p.tile([P, C], f32, tag="tmp")
                    nc.vector.tensor_tensor(out=tmp[:], in0=ps[:], in1=ads0[:], op=ALU.mult)
                    nc.vector.tensor_reduce(
                        out=adtab[0][:, j * 4 : (j + 1) * 4],
                        in_=tmp[:].rearrange("p (h c) -> p h c", h=HEADS),
                        axis=mybir.AxisListType.X, op=ALU.add)

            # ================= GAT layers =================
            for l in range(L):
                with tc.tile_pool(name=f"b{l}", bufs=2) as bp, \
                     tc.tile_pool(name=f"b{l}g", bufs=8) as bg, \
                     tc.tile_pool(name=f"b{l}ps", bufs=2, space="PSUM") as bps, \
                     tc.tile_pool(name=f"b{l}pso", bufs=1, space="PSUM") as bpso:
                    brep = bp.tile([P, C], f32, tag="brep", bufs=1)
                    nc.sync.dma_start(out=brep[:], in_=d_brep[l][:, :])
                    if l < L - 1:
                        Wn = bp.tile([P, 4, C], f32r, tag="wn", bufs=1)
                        for kc in range(4):
                            nc.sync.dma_start(out=Wn[:, kc, :], in_=d_W[l + 1][kc * P : (kc + 1) * P, :])
                        asrn = bp.tile([P, C], f32, tag="asrn", bufs=1)
                        nc.sync.dma_start(out=asrn[:], in_=d_asrc[l + 1][:, :])
                        adsn = bp.tile([P, C], f32, tag="adsn", bufs=1)
                        nc.sync.dma_start(out=adsn[:], in_=d_adst[l + 1][:, :])
                    for j in range(NTILES_OWN):
                        ps_out = bpso.tile([P, C], f32, tag="pso")
                        ps_den = bpso.tile([P, C], f32, tag="psd")
                        for c in range(nct):
                            col = j * nct + c
                            hg = bg.tile([P, DH], f32r, tag="hg")
                            nc.gpsimd.indirect_dma_start(
                                out=hg[:], out_offset=None, in_=d_haug[l][:, :],
                                in_offset=bass.IndirectOffsetOnAxis(ap=srcidx[:, col : col + 1], axis=0))
                            st = bp.tile([P, P], f32r, tag="st")
                            nc.vector.tensor_tensor(
                                out=st[:], in0=iota_f[:],
                                in1=dlt[:, col : col + 1].to_broadcast([P, P]), op=ALU.is_equal)
                            s_ps = bps.tile([P, P], f32r, tag="sps")
                            nc.tensor.transpose(out=s_ps[:], in_=st[:], identity=ident[:])
                            s_sb = bp.tile([P, P], f32r, tag="ssb")
                            nc.vector.tensor_copy(out=s_sb[:], in_=s_ps[:])
                            ade = bps.tile([P, C], f32, tag="ade")
                            nc.tensor.matmul(out=ade[:, 0:4], lhsT=s_sb[:],
                                             rhs=adtab[l][:, j * 4 : (j + 1) * 4],
                                             start=True, stop=True)
                            t1 = bp.tile([P, 4], f32, tag="t1")
                            nc.vector.tensor_tensor(out=t1[:], in0=hg[:, C : C + 4], in1=ade[:, 0:4], op=ALU.add)
                            t2 = bp.tile([P, 4], f32, tag="t2")
                            nc.vector.tensor_scalar(out=t2[:], in0=t1[:], scalar1=0.2,
                                                    scalar2=None, op0=ALU.mult)
                            t3 = bp.tile([P, 4], f32, tag="t3")
                            nc.vector.tensor_tensor(out=t3[:], in0=t1[:], in1=t2[:], op=ALU.max)
                            ex = bp.tile([P, 4], f32r, tag="ex")
                            nc.scalar.activation(out=ex[:], in_=t3[:], func=AF.Exp)
                            hgw = bp.tile([P, C], f32r, tag="hgw")
                            for h in range(HEADS):
                                nc.vector.tensor_tensor(
                                    out=hgw[:, h * HID : (h + 1) * HID],
                                    in0=hg[:, h * HID : (h + 1) * HID],
                                    in1=ex[:, h : h + 1].to_broadcast([P, HID]), op=ALU.mult)
                            nc.tensor.matmul(out=ps_out[:], lhsT=st[:], rhs=hgw[:],
                                             start=(c == 0), stop=(c == nct - 1))
                            nc.tensor.matmul(out=ps_den[:, 0:4], lhsT=st[:], rhs=ex[:],
                                             start=(c == 0), stop=(c == nct - 1))
                        # -------- epilogue for node tile j --------
                        den = bp.tile([P, 4], f32, tag="den")
                        nc.vector.tensor_scalar(out=den[:], in0=ps_den[:, 0:4], scalar1=1e-30,
                                                scalar2=None, op0=ALU.max)
                        rec = bp.tile([P, 4], f32, tag="rec")
                        nc.vector.reciprocal(out=rec[:], in_=den[:])
                        xl = bp.tile([P, C], f32, tag="xl")
                        for h in range(HEADS):
                            nc.vector.tensor_scalar(
                                out=xl[:, h * HID : (h + 1) * HID],
                                in0=ps_out[:, h * HID : (h + 1) * HID],
                                scalar1=rec[:, h : h + 1], scalar2=None, op0=ALU.mult)
                        nc.vector.tensor_tensor(out=xl[:], in0=xl[:], in1=brep[:], op=ALU.add)
                        xr = bp.tile([P, C], f32r, tag="xr")
                        nc.scalar.activation(out=xr[:], in_=xl[:], func=AF.Relu)
                        xrf = bp.tile([P, C], f32, tag="xrf")
                        nc.scalar.activation(out=xrf[:], in_=xl[:], func=AF.Relu)
                        nc.sync.dma_start(out=d_x[l][j * P : (j + 1) * P, :], in_=xrf[:])
                        tsbs = []
                        for kc in range(4):
                            tp = bps.tile([P, P], f32r, tag="sps")
                            nc.tensor.transpose(out=tp[:], in_=xr[:, kc * P : (kc + 1) * P], identity=ident[:])
                            tsb = bp.tile([P, P], f32r, tag=f"tsb{kc}")
                            nc.vector.tensor_copy(out=tsb[:], in_=tp[:])
                            nc.sync.dma_start(
                                out=d_xt[l][kc * P : (kc + 1) * P, j * P : (j + 1) * P], in_=tsb[:])
                            tsbs.append(tsb)
                        if l < L - 1:
                            psA = bpso.tile([P, C], f32, tag="psA")
                            for kc in range(4):
                                nc.tensor.matmul(out=psA[:], lhsT=tsbs[kc][:], rhs=Wn[:, kc, :],
                                                 start=(kc == 0), stop=(kc == 3))
                            hsh = bp.tile([P, DH], f32r, tag="hsh")
                            nc.scalar.copy(out=hsh[:, 0:C], in_=psA[:])
                            tmp = bp.tile([P, C], f32, tag="tmp")
                            nc.vector.tensor_tensor(out=tmp[:], in0=psA[:], in1=asrn[:], op=ALU.mult)
                            nc.vector.tensor_reduce(
                                out=hsh[:, C : C + 4],
                                in_=tmp[:].rearrange("p (h c) -> p h c", h=HEADS),
                                axis=mybir.AxisListType.X, op=ALU.add)
                            nc.sync.dma_start(out=d_hsh[l + 1][j * P : (j + 1) * P, :], in_=hsh[:])
                            tmp2 = bp.tile([P, C], f32, tag="tmp2")
                            nc.vector.tensor_tensor(out=tmp2[:], in0=psA[:], in1=adsn[:], op=ALU.mult)
                            nc.vector.tensor_reduce(
                                out=adtab[l + 1][:, j * 4 : (j + 1) * 4],
                                in_=tmp2[:].rearrange("p (h c) -> p h c", h=HEADS),
                                axis=mybir.AxisListType.X, op=ALU.add)
                    if l < L - 1:
                        nc.gpsimd.collective_compute(
                            "AllGather", ALU.bypass, replica_groups=RG,
                            ins=[d_hsh[l + 1].ap()], outs=[d_haug[l + 1].ap()])

            # ================= LSTM (both dirs) + scores =================
            with tc.tile_pool(name="lstm", bufs=1) as lp, \
                 tc.tile_pool(name="lstm1", bufs=1) as lp1, \
                 tc.tile_pool(name="lstm2", bufs=2) as lp2, \
                 tc.tile_pool(name="lstm3", bufs=3) as lp3, \
                 tc.tile_pool(name="lps", bufs=3, space="PSUM") as lps, \
                 tc.tile_pool(name="lps2", bufs=1, space="PSUM") as lps2:
                bsum = lp.tile([P, 48], f32, tag="bsum")
                nc.sync.dma_start(out=bsum[:], in_=d_bsum[:, :])
                attw = lp.tile([P, 12], f32r, tag="attw")
                nc.sync.dma_start(out=attw[:], in_=d_attw[:, :])
                for dire in range(2):
                    cst = [lp.tile([P, NPCP], f32, tag=f"c{j}", name=f"c{j}") for j in range(6)]
                    for j in range(6):
                        nc.vector.memset(cst[j][:], 0.0)
                    h_prev = [None] * 6
                    for step in range(3):
                        t = step if dire == 0 else 2 - step
                        xtt = lp.tile([P, 4, NPCP], f32r, tag="xtt")
                        for kc in range(4):
                            nc.sync.dma_start(out=xtt[:, kc, :], in_=d_xt[t][kc * P : (kc + 1) * P, 0:NPCP])
                        h_new = [None] * 6
                        scp = [lps2.tile([1, 512], f32, tag=f"scp{b}", name=f"scp{b}") for b in range(3)]
                        for j in range(6):
                            gas = []
                            for gate in range(4):
                                gt_row = gate * 6 + j
                                wih = lp3.tile([P, 4, P], f32r, tag="wih")
                                nc.sync.dma_start(
                                    out=wih[:],
                                    in_=d_wih[dire][:, gt_row * P : (gt_row + 1) * P]
                                    .rearrange("(k p) g -> p k g", p=P))
                                if step > 0:
                                    whh = lp3.tile([P, 6, P], f32r, tag="whh")
                                    nc.sync.dma_start(
                                        out=whh[:],
                                        in_=d_whh[dire][:, gt_row * P : (gt_row + 1) * P]
                                        .rearrange("(k p) g -> p k g", p=P))
                                ga = lp1.tile([P, NPCP], f32, tag=f"ga{gate}")
                                for b, (b0, bw) in enumerate(BLKS):
                                    gps = lps.tile([P, 512], f32, tag="gps")
                                    for kc in range(4):
                                        nc.tensor.matmul(
                                            out=gps[:, 0:bw], lhsT=wih[:, kc, :],
                                            rhs=xtt[:, kc, b0 : b0 + bw],
                                            start=(kc == 0),
                                            stop=(kc == 3 and step == 0))
                                    if step > 0:
                                        for kc in range(6):
                                            nc.tensor.matmul(
                                                out=gps[:, 0:bw], lhsT=whh[:, kc, :],
                                                rhs=h_prev[kc][:, b0 : b0 + bw],
                                                start=False, stop=(kc == 5))
                                    nc.scalar.activation(
                                        out=ga[:, b0 : b0 + bw], in_=gps[:, 0:bw],
                                        func=(AF.Tanh if gate == 2 else AF.Sigmoid),
                                        bias=bsum[:, dire * 24 + gt_row : dire * 24 + gt_row + 1])
                                gas.append(ga)
                            # c/h update for hl-tile j
                            tmp1 = lp1.tile([P, NPCP], f32, tag="tmp1")
                            nc.vector.tensor_tensor(out=tmp1[:], in0=gas[0][:], in1=gas[2][:], op=ALU.mult)
                            nc.vector.tensor_tensor(out=cst[j][:], in0=cst[j][:], in1=gas[1][:], op=ALU.mult)
                            nc.vector.tensor_tensor(out=cst[j][:], in0=cst[j][:], in1=tmp1[:], op=ALU.add)
                            tmp2 = lp1.tile([P, NPCP], f32, tag="tmp2")
                            nc.scalar.activation(out=tmp2[:], in_=cst[j][:], func=AF.Tanh)
                            hj = lp2.tile([P, NPCP], f32r, tag=f"h{j}")
                            nc.vector.tensor_tensor(out=hj[:], in0=tmp2[:], in1=gas[3][:], op=ALU.mult)
                            h_new[j] = hj
                            for b, (b0, bw) in enumerate(BLKS):
                                nc.tensor.matmul(
                                    out=scp[b][:, 0:bw], lhsT=attw[:, dire * 6 + j : dire * 6 + j + 1],
                                    rhs=hj[:, b0 : b0 + bw], start=(j == 0), stop=(j == 5))
                        h_prev = h_new
                        for b, (b0, bw) in enumerate(BLKS):
                            nc.vector.tensor_tensor(
                                out=scores[0:1, t * NPCP + b0 : t * NPCP + b0 + bw],
                                in0=scores[0:1, t * NPCP + b0 : t * NPCP + b0 + bw],
                                in1=scp[b][:, 0:bw], op=ALU.add)

            # ================= JK attention + pooling =================
            with tc.tile_pool(name="jk", bufs=2) as jp, \
                 tc.tile_pool(name="jkpso", bufs=1, space="PSUM") as jpso:
                nc.sync.dma_start(
                    out=d_scores[:, :], in_=scores[0:1, :].rearrange("o (t n) -> (o t) n", t=L))
                poolmat = jp.tile([P, NTILES_OWN, G], f32r, tag="pm")
                nc.sync.dma_start(out=poolmat[:], in_=d_poolmat[:, :, :])
                pool_ps = jpso.tile([G, C], f32, tag="poolps")
                for j in range(NTILES_OWN):
                    sc = jp.tile([P, 3], f32, tag="sc")
                    nc.sync.dma_start(
                        out=sc[:], in_=d_scores[:, j * P : (j + 1) * P].rearrange("t p -> p t"))
                    ex3 = jp.tile([P, 3], f32, tag="ex3")
                    nc.scalar.activation(out=ex3[:], in_=sc[:], func=AF.Exp)
                    s1 = jp.tile([P, 1], f32, tag="s1")
                    nc.vector.tensor_reduce(out=s1[:], in_=ex3[:], axis=mybir.AxisListType.X, op=ALU.add)
                    rec = jp.tile([P, 1], f32, tag="rec1")
                    nc.vector.reciprocal(out=rec[:], in_=s1[:])
                    alpha = jp.tile([P, 3], f32, tag="alpha")
                    nc.vector.tensor_scalar(out=alpha[:], in0=ex3[:], scalar1=rec[:, 0:1],
                                            scalar2=None, op0=ALU.mult)
                    acc = None
                    for t in range(3):
                        xlt = jp.tile([P, C], f32, tag=f"xlt{t}")
                        nc.sync.dma_start(out=xlt[:], in_=d_x[t][j * P : (j + 1) * P, :])
                        w = jp.tile([P, C], f32 if t < 2 else f32r, tag=f"w{t}")
                        nc.vector.tensor_scalar(out=w[:], in0=xlt[:], scalar1=alpha[:, t : t + 1],
                                                scalar2=None, op0=ALU.mult)
                        if t == 0:
                            acc = w
                        elif t == 1:
                            nc.vector.tensor_tensor(out=acc[:], in0=acc[:], in1=w[:], op=ALU.add)
                        else:
                            xjk = jp.tile([P, C], f32r, tag="xjk")
                            nc.vector.tensor_tensor(out=xjk[:], in0=acc[:], in1=w[:], op=ALU.add)
                    nc.tensor.matmul(out=pool_ps[:], lhsT=poolmat[:, j, :], rhs=xjk[:],
                                     start=(j == 0), stop=(j == NTILES_OWN - 1))
                pool_sb = jp.tile([G, C], f32, tag="poolsb")
                nc.vector.tensor_copy(out=pool_sb[:], in_=pool_ps[:])
                nc.sync.dma_start(out=d_poolin[:, :], in_=pool_sb[:])
                nc.gpsimd.collective_compute(
                    "AllReduce", ALU.add, replica_groups=RG,
                    ins=[d_poolin.ap()], outs=[d_pooled.ap()])

            # ================= MLP =================
            with tc.tile_pool(name="mlp", bufs=1) as mp, \
                 tc.tile_pool(name="mlpps", bufs=1, space="PSUM") as mps:
                fc1 = mp.tile([P, 4, C], f32r, tag="fc1")
                fc2 = mp.tile([P, 4, C], f32r, tag="fc2")
                for kc in range(4):
                    nc.sync.dma_start(out=fc1[:, kc, :], in_=d_fc1[kc * P : (kc + 1) * P, :])
                    nc.sync.dma_start(out=fc2[:, kc, :], in_=d_fc2[kc * P : (kc + 1) * P, :])
                fc3 = mp.tile([P, 4, OUT], f32r, tag="fc3")
                for kc in range(4):
                    nc.sync.dma_start(out=fc3[:, kc, :], in_=d_fc3[kc * P : (kc + 1) * P, :])
                fcb = mp.tile([P, 8], f32, tag="fcb")
                nc.sync.dma_start(out=fcb[:], in_=d_fcb[:, :])
                fc3b = mp.tile([OUT, 1], f32, tag="fc3b")
                nc.sync.dma_start(out=fc3b[:], in_=d_fc3b[:, :])
                plf = mp.tile([G, C], f32, tag="plf")
                nc.sync.dma_start(out=plf[:], in_=d_pooled[:, :])
                pl = mp.tile([G, C], f32r, tag="pl")
                nc.vector.tensor_copy(out=pl[:], in_=plf[:])
                gT = []
                for kc in range(4):
                    tp = mps.tile([P, G], f32r, tag="mtp")
                    nc.tensor.transpose(out=tp[:, 0:G], in_=pl[0:G, kc * P : (kc + 1) * P],
                                        identity=ident[0:G, 0:G])
                    tsb = mp.tile([P, G], f32r, tag=f"gT{kc}")
                    nc.vector.tensor_copy(out=tsb[:], in_=tp[:, 0:G])
                    gT.append(tsb)
                h1 = []
                for co in range(4):
                    ps = mps.tile([P, G], f32, tag="mps1")
                    for kc in range(4):
                        nc.tensor.matmul(out=ps[:, 0:G], lhsT=fc1[:, kc, co * P : (co + 1) * P],
                                         rhs=gT[kc][:, 0:G], start=(kc == 0), stop=(kc == 3))
                    t = mp.tile([P, G], f32r, tag=f"h1{co}")
                    nc.scalar.activation(out=t[:], in_=ps[:, 0:G], func=AF.Relu,
                                         bias=fcb[:, co : co + 1])
                    h1.append(t)
                h2 = []
                for co in range(4):
                    ps = mps.tile([P, G], f32, tag="mps2")
                    for kc in range(4):
                        nc.tensor.matmul(out=ps[:, 0:G], lhsT=fc2[:, kc, co * P : (co + 1) * P],
                                         rhs=h1[kc][:, 0:G], start=(kc == 0), stop=(kc == 3))
                    t = mp.tile([P, G], f32r, tag=f"h2{co}")
                    nc.scalar.activation(out=t[:], in_=ps[:, 0:G], func=AF.Relu,
                                         bias=fcb[:, 4 + co : 5 + co])
                    h2.append(t)
                ps = mps.tile([P, G], f32, tag="mps3")
                for kc in range(4):
                    nc.tensor.matmul(out=ps[0:OUT, 0:G], lhsT=fc3[:, kc, :], rhs=h2[kc][:, 0:G],
                                     start=(kc == 0), stop=(kc == 3))
                osb = mp.tile([OUT, G], f32, tag="osb")
                nc.scalar.activation(out=osb[:], in_=ps[0:OUT, 0:G], func=AF.Identity,
                                     bias=fc3b[:, 0:1])
                nc.sync.dma_start(out=d_out[:, :], in_=osb[:])

    nc.compile()
    return nc


def build_in_maps(inputs, nct, srcidx, dlt, poolmat):
    inputs = {k: np.asarray(v) for k, v in inputs.items()}
    x = inputs["x"].astype(np.float32)
    xpad = np.zeros((NT, IN_C), np.float32)
    idx = np.arange(N)
    xpad[(idx // NPC) * NPCP + (idx % NPC)] = x
    xT = np.ascontiguousarray(xpad.T)  # [128, NT]

    shared = {
        "xT": xT,
        "W0d": inputs["W0"].astype(np.float32),
        "W1d": inputs["W1"].astype(np.float32),
        "W2d": inputs["W2"].astype(np.float32),
        "fc1W": inputs["fc1_W"].astype(np.float32),
        "fc2W": inputs["fc2_W"].astype(np.float32),
        "fc3W": inputs["fc3_W"].astype(np.float32),
        "fc3b": inputs["fc3_b"].reshape(OUT, 1).astype(np.float32),
    }
    for l in range(L):
        shared[f"asrc{l}"] = np.tile(inputs[f"asrc{l}"].reshape(1, C), (P, 1)).astype(np.float32)
        shared[f"adst{l}"] = np.tile(inputs[f"adst{l}"].reshape(1, C), (P, 1)).astype(np.float32)
        shared[f"brep{l}"] = np.tile(inputs[f"b{l}"].reshape(1, C), (P, 1)).astype(np.float32)
    for i, d in enumerate("fr"):
        shared[f"WihT_{d}"] = np.ascontiguousarray(inputs[f"Wih_{d}"].T).astype(np.float32)
        shared[f"WhhT_{d}"] = np.ascontiguousarray(inputs[f"Whh_{d}"].T).astype(np.float32)
    bsum = np.zeros((P, 48), np.float32)
    for i, d in enumerate("fr"):
        bs = (inputs[f"bih_{d}"] + inputs[f"bhh_{d}"]).astype(np.float32)  # [3072]
        bsum[:, i * 24 : (i + 1) * 24] = bs.reshape(24, P).T
    shared["bsum"] = bsum
    attw = np.zeros((P, 12), np.float32)
    aw = inputs["att_w"].astype(np.float32)
    attw[:, 0:6] = aw[0:HL].reshape(6, P).T
    attw[:, 6:12] = aw[HL:].reshape(6, P).T
    shared["attw"] = attw
    fcb = np.zeros((P, 8), np.float32)
    fcb[:, 0:4] = inputs["fc1_b"].reshape(4, P).T
    fcb[:, 4:8] = inputs["fc2_b"].reshape(4, P).T
    shared["fcb"] = fcb

    in_maps = []
    for k in range(NCORES):
        m = dict(shared)
        m["xTown"] = np.ascontiguousarray(
            xT[:, k * NPCP : (k + 1) * NPCP].reshape(P, NTILES_OWN, P))
        m["srcidx"] = srcidx[k]
        m["dlt"] = dlt[k]
        m["poolmat"] = poolmat[k]
        in_maps.append(m)
    return in_maps


def get_kernel(nct):
    if nct not in _CACHE:
        _CACHE[nct] = build_nc(nct)
    return _CACHE[nct]


def kernel(**inputs):
    nct, srcidx, dlt, poolmat = build_tables(inputs["edge_index"], inputs["batch"])
    nc = get_kernel(nct)
    in_maps = build_in_maps(inputs, nct, srcidx, dlt, poolmat)
    from concourse.bass_utils import run_bass_kernel_spmd

    res = run_bass_kernel_spmd(nc, in_maps, core_ids=list(range(NCORES)))
    out_T = res.results[0]["out_T"]
    return np.ascontiguousarray(out_T.T.astype(np.float32))
